# revision 24
# baseline (speedup 1.0000x reference)
"""HQQ int4 weight-only quantized linear for TRN2, 8-core tensor-parallel.

out[M, N] = x[M, K] @ dequant(W_q[N, K]).T
  dequant: w[n, k] = (q[n, k] - 8) * scales[n, k//128] + zeros[n, k//128]

Sharding: column-parallel over N (out_features) across 8 NeuronCores;
x replicated; outputs concatenated on host. No collectives.

Mixed-precision K-split (device does only matmuls; all dequant + zero-point
compensation is host-side):
  - NB=18 quant groups matmul'd in bf16: w_bf = (q-8)*s*512 (bf16), x as-is.
  - NF8=14 groups in fp8 e4m3 DoubleRow (2 groups per instruction, 2x PE
    throughput): w8 = e4m3((q-8)*s*32), x8 = e4m3(x*16) -> products carry
    the same x512 scale, so both parts accumulate in one PSUM bank.
  - PSUM evicted as raw f32; host computes out = bf16(psum/512 + seed) where
    seed = R@ (512 z).T plus group-mean quantization-error corrections
    (all exact f32 host math).
Per 128-row m-subtile: 3 n-tiles x (18 bf16 + 7 DoubleRow) = 75 matmuls
(25 "units" of 1376 rows vs 33 for all-bf16 with on-device seeds).
"""

import sys

import numpy as np
import ml_dtypes

M = 4096
K = 4096
N = 11008
GROUP = 128
N_CORES = 8
N_SHARD = N // N_CORES  # 1376
NG = K // GROUP  # 32 quant groups
NB = 16  # groups done in bf16
NF8 = NG - NB  # groups done in fp8 DoubleRow
NPAIR = NF8 // 2
M_PANEL = 256
SW = 32.0  # fp8 weight scale
SX = 16.0  # fp8 x scale
SC = SW * SX  # common PSUM scale (512)
BF16 = ml_dtypes.bfloat16
E4M3 = ml_dtypes.float8_e4m3


def _install_axon_hooks_shim():
    """antenv.axon_hooks is missing from this image; run_bass_kernel_spmd
    imports it when tracing is requested (e.g. BASS_TRACE=1). Provide the
    same ctypes-based hook trn_boot would have registered."""
    import types

    try:
        import antenv.axon_hooks  # noqa: F401

        return
    except ImportError:
        pass
    try:
        import antenv
        from trn_agent_boot.trn_boot import _ntff_profile_via_ctypes

        hook = _ntff_profile_via_ctypes("/opt/axon/libaxon_pjrt.so")
        mod = types.ModuleType("antenv.axon_hooks")
        mod._hook = hook
        mod.get_axon_ntff_profile_hook = lambda: mod._hook

        def _set(h):
            mod._hook = h

        mod.set_axon_ntff_profile_hook = _set
        sys.modules["antenv.axon_hooks"] = mod
        antenv.axon_hooks = mod
    except Exception:
        pass


def build_bass(m=M, n_shard=N_SHARD, nb=NB, npair=NPAIR, compile=True):
    import concourse.mybir as mybir
    import concourse.tile as tile
    from concourse import bacc

    P = 128
    MP = M_PANEL
    f32 = mybir.dt.float32
    bf16 = mybir.dt.bfloat16
    fp8 = mybir.dt.float8e4
    n_panels = m // MP
    nsub = MP // P  # m-subtiles per panel (2)

    nc = bacc.Bacc("TRN2", target_bir_lowering=False, debug=False)
    xb4 = nc.dram_tensor("xb4", [n_panels, P, nb, MP], bf16, kind="ExternalInput")
    xf4 = nc.dram_tensor("xf4", [n_panels, P, npair, 2, MP], fp8, kind="ExternalInput")
    wb = nc.dram_tensor("wb", [nb, P, n_shard], bf16, kind="ExternalInput")
    wf = nc.dram_tensor("wf", [npair, P, 2, n_shard], fp8, kind="ExternalInput")
    out = nc.dram_tensor("out", [m, n_shard], f32, kind="ExternalOutput")

    n_tiles = []
    st = 0
    while st < n_shard:
        nf = min(512, n_shard - st)
        n_tiles.append((st, nf))
        st += nf

    with tile.TileContext(nc) as tc:
        with (
            tc.tile_pool(name="warm", bufs=1) as warm_pool,
            tc.tile_pool(name="wbp", bufs=nb) as wb_pool,
            tc.tile_pool(name="wfp", bufs=npair) as wf_pool,
            tc.tile_pool(name="xbp", bufs=3) as xb_pool,
            tc.tile_pool(name="xfp", bufs=2) as xf_pool,
            tc.tile_pool(name="osb", bufs=2) as osb_pool,
            tc.tile_pool(name="pswm", bufs=1, space="PSUM") as pswm_pool,
            tc.tile_pool(name="psum", bufs=6, space="PSUM") as psum_pool,
        ):
            # ---- HAM warm-up: dummy matmuls with no DMA deps keep the PE
            # busy during the initial weight/x DMA wait so the clock gate is
            # at K=8/8 (2.4 GHz) when real matmuls start ----
            wrm_w = warm_pool.tile([P, P], bf16, tag="wrmw")
            wrm_x = warm_pool.tile([P, 512], bf16, tag="wrmx")
            nc.vector.memset(wrm_w[:], 0.0)
            nc.vector.memset(wrm_x[:], 0.0)
            wrm_ps = pswm_pool.tile([P, 512], f32, tag="wrmp")
            NWARM = 20
            for i in range(NWARM):
                nc.tensor.matmul(
                    wrm_ps[:], wrm_w[:], wrm_x[:],
                    start=(i == 0), stop=(i == NWARM - 1),
                )

            xb_tiles = {}
            xf_tiles = {}

            def load_panel(mp):
                xb_tiles[mp] = xb_pool.tile([P, nb, MP], bf16, tag="xb", name=f"xb{mp}")
                nc.sync.dma_start(xb_tiles[mp][:], xb4[mp])
                xf_tiles[mp] = xf_pool.tile(
                    [P, npair, 2, MP], fp8, tag="xf", name=f"xf{mp}"
                )
                nc.sync.dma_start(xf_tiles[mp][:], xf4[mp])

            # ---- DMA issue order + matching k-sweep order. Startup is
            # aggregate-HBM-bound (~10 MB of weights + panel-0 x), so the
            # sweep alternates fp8 pairs (fed by the scalar queue) with even
            # bf16 groups (fed by the sync queue) — each queue then only has
            # to sustain one 0.35 MB tile per 2.29 us — and consumes the odd
            # bf16 groups (queued behind the fp8 weights on scalar) last.
            # step schedule: two fp8 steps lead (xb0 is behind xf0 on sync),
            # then fp8/bf16-even alternate, odd bf16 groups last
            steps = [("f", 0), ("f", 1), ("b", 0)]
            for i in range(2, npair):
                steps += [("f", i), ("b", 2 * (i - 1))]
            steps += [("b", 14)]
            steps += [("b", g) for g in range(1, nb, 2)]
            assert len(steps) == nb + npair
            assert sorted(i for k, i in steps if k == "b") == list(range(nb))
            assert sorted(i for k, i in steps if k == "f") == list(range(npair))

            # Per-queue delivery schedule, derived from measured queue rates
            # (sync ~0.21 MB/us from ~8.6us, scalar ~0.24 MB/us from ~10.5us,
            # consumption one 0.35 MB tile per 1.15us from ~14.5us): scalar
            # covers steps {0,1,2} then every other step; sync carries the
            # panel-0 x tensors first, then the remaining steps' weights.
            q10_steps = {0, 1, 2} | set(range(4, len(steps), 2))

            xf_tiles[0] = xf_pool.tile([P, npair, 2, MP], fp8, tag="xf", name="xf0")
            nc.sync.dma_start(xf_tiles[0][:], xf4[0])
            xb_tiles[0] = xb_pool.tile([P, nb, MP], bf16, tag="xb", name="xb0")
            nc.sync.dma_start(xb_tiles[0][:], xb4[0])

            wb_tiles = [None] * nb
            wf_tiles = [None] * npair
            for ring_is_q10 in (True, False):
                for si, (kind, idx) in enumerate(steps):
                    if (si in q10_steps) != ring_is_q10:
                        continue
                    ring = nc.scalar if ring_is_q10 else nc.sync
                    if kind == "f":
                        wt = wf_pool.tile(
                            [P, 2, n_shard], fp8, tag="wf", name=f"wf{idx}"
                        )
                        ring.dma_start(wt[:], wf[idx])
                        wf_tiles[idx] = wt
                    else:
                        wt = wb_pool.tile(
                            [P, n_shard], bf16, tag="wb", name=f"wb{idx}"
                        )
                        ring.dma_start(wt[:], wb[idx])
                        wb_tiles[idx] = wt

            def mm_sweep_one(pss, mp, ms, si):
                """One k-step (steps[si]) across the 3 n-tiles."""
                kind, idx = steps[si]
                first, last = si == 0, si == len(steps) - 1
                if kind == "f":
                    lhsT = xf_tiles[mp][:, idx, :, ms * P : (ms + 1) * P]
                    for j, (st, nf) in enumerate(n_tiles):
                        nc.tensor.matmul(
                            pss[j],
                            lhsT,
                            wf_tiles[idx][:, :, st : st + nf],
                            start=first,
                            stop=last,
                            perf_mode=mybir.MatmulPerfMode.DoubleRow,
                        )
                else:
                    lhsT = xb_tiles[mp][:, idx, ms * P : (ms + 1) * P]
                    for j, (st, nf) in enumerate(n_tiles):
                        nc.tensor.matmul(
                            pss[j],
                            lhsT,
                            wb_tiles[idx][:, st : st + nf],
                            start=first,
                            stop=last,
                        )

            def evict(psums, ms_abs):
                osb = osb_pool.tile([P, n_shard], f32, tag="osb")
                for j, (st, nf) in enumerate(n_tiles):
                    nc.any.tensor_copy(osb[:, st : st + nf], psums[j])
                m0 = ms_abs * P
                nc.scalar.dma_start(out[m0 : m0 + P, :], osb[:])

            nsteps = nb + npair

            def emit_panel_k_outer(mp):
                # both m-subtiles' k-sweeps interleaved: 6 open psum banks;
                # matmuls chase the weight DMAs on the first panels.
                pss = []
                for ms in range(nsub):
                    row = []
                    for j, (st, nf) in enumerate(n_tiles):
                        ps = psum_pool.tile([P, 512], f32, tag="ps", name="psA")[:, :nf]
                        row.append(ps)
                    pss.append(row)
                for g in range(nsteps):
                    for ms in range(nsub):
                        mm_sweep_one(pss[ms], mp, ms, g)
                for ms in range(nsub):
                    evict(pss[ms], mp * nsub + ms)

            def emit_panel_ms_inner(mp):
                for ms in range(nsub):
                    psums = []
                    for j, (st, nf) in enumerate(n_tiles):
                        ps = psum_pool.tile([P, 512], f32, tag="ps", name="psB")[:, :nf]
                        psums.append(ps)
                    for g in range(nsteps):
                        mm_sweep_one(psums, mp, ms, g)
                    evict(psums, mp * nsub + ms)

            def emit_last_ms_j_outer(mp, ms):
                # j-outer so each n-tile's k-sweep finishes early and its
                # evict + out-DMA overlap the remaining n-tiles' matmuls —
                # shortens the end-of-kernel tail.
                xb_t, xf_t = xb_tiles[mp], xf_tiles[mp]
                osb = osb_pool.tile([P, n_shard], f32, tag="osb", name="osbL")
                m0 = (mp * nsub + ms) * P
                for j, (st, nf) in enumerate(n_tiles):
                    ps = psum_pool.tile([P, 512], f32, tag="ps", name="psC")[:, :nf]
                    for g in range(nb):
                        nc.tensor.matmul(
                            ps,
                            xb_t[:, g, ms * P : (ms + 1) * P],
                            wb_tiles[g][:, st : st + nf],
                            start=(g == 0),
                            stop=False,
                        )
                    for p in range(npair):
                        nc.tensor.matmul(
                            ps,
                            xf_t[:, p, :, ms * P : (ms + 1) * P],
                            wf_tiles[p][:, :, st : st + nf],
                            start=False,
                            stop=(p == npair - 1),
                            perf_mode=mybir.MatmulPerfMode.DoubleRow,
                        )
                    nc.any.tensor_copy(osb[:, st : st + nf], ps)
                    nc.scalar.dma_start(
                        out[m0 : m0 + P, st : st + nf], osb[:, st : st + nf]
                    )

            for mp in range(n_panels):
                if mp > 0:
                    load_panel(mp)
                if mp < 2:
                    emit_panel_k_outer(mp)
                elif mp < n_panels - 1:
                    emit_panel_ms_inner(mp)
                else:
                    # last panel: normal first subtile, j-outer last subtile
                    psums = []
                    for j, (st, nf) in enumerate(n_tiles):
                        ps = psum_pool.tile([P, 512], f32, tag="ps", name="psB")[:, :nf]
                        psums.append(ps)
                    for g in range(nsteps):
                        mm_sweep_one(psums, mp, 0, g)
                    evict(psums, mp * nsub)
                    emit_last_ms_j_outer(mp, 1)

    if compile:
        nc.compile()
    return nc


def host_prep(x, W_q, scales, zeros):
    """Host-side layout/quantization prep. Returns per-core input maps'
    building blocks plus the f32 seed (zero-point + quant-error-mean
    compensation) to add to the device output."""
    n_panels = M // M_PANEL
    x = np.asarray(x)
    xf32 = x.astype(np.float32)
    sf = np.asarray(scales).astype(np.float32)
    zf = np.asarray(zeros).astype(np.float32)

    # dequantized (scaled) weight without zeros, transposed to [K, N]
    q = np.asarray(W_q).astype(np.float32) - 8.0  # [N, K]
    w_sT = (q.reshape(N, NG, GROUP) * sf[:, :, None]).reshape(N, K).T  # [K, N]
    w_sT = np.ascontiguousarray(w_sT)

    kb = NB * GROUP  # bf16 k-columns: [0, kb); fp8: [kb, K)
    w_bfT = (w_sT[:kb] * SC).astype(BF16)  # [kb, N]
    w_f8T = (w_sT[kb:] * SW).astype(E4M3)  # [K-kb, N]

    # x tiles: [panel, p, group, m_in_panel]
    xt = np.ascontiguousarray(
        x.reshape(n_panels, M_PANEL, NG, GROUP).transpose(0, 3, 2, 1)
    )  # [np, 128, ng, MP] bf16
    xb4 = np.ascontiguousarray(xt[:, :, :NB, :])
    x8 = (xf32 * SX).astype(E4M3)  # [M, K]
    x8t = np.ascontiguousarray(
        x8.reshape(n_panels, M_PANEL, NG, GROUP)
        .transpose(0, 3, 2, 1)[:, :, NB:, :]
        .reshape(n_panels, GROUP, NPAIR, 2, M_PANEL)
    )

    # weight tensors in device layout
    wb_full = np.ascontiguousarray(w_bfT.reshape(NB, GROUP, N))  # [nb, 128, N]
    wf_full = np.ascontiguousarray(
        w_f8T.reshape(NPAIR, 2, GROUP, N).transpose(0, 2, 1, 3)
    )  # [npair, 128, 2, N]

    # ---- host seed: zero-point compensation + quant-error group means ----
    R = xf32.reshape(M, NG, GROUP).sum(-1)  # [M, ng] exact group sums
    e_x = xf32 - x8.astype(np.float32) / SX  # x quant residual
    Ex = e_x.reshape(M, NG, GROUP).sum(-1)[:, NB:]  # [M, nf8]

    w_q_errT = w_sT[kb:] - w_f8T.astype(np.float32) / SW  # [kf, N]
    ebar8 = w_q_errT.reshape(NF8, GROUP, N).mean(1).T  # [N, nf8]
    w_bf_errT = w_sT[:kb] - w_bfT.astype(np.float32) / SC  # [kb, N]
    ebarb = w_bf_errT.reshape(NB, GROUP, N).mean(1).T  # [N, nb]
    wbar8 = w_sT[kb:].reshape(NF8, GROUP, N).mean(1).T  # [N, nf8]

    zc = zf.copy()  # [N, ng]
    zc[:, :NB] += ebarb
    zc[:, NB:] += ebar8
    seed = R @ zc.T + Ex @ (wbar8 - ebar8).T  # [M, N] f32
    return xb4, x8t, wb_full, wf_full, seed


_NC_CACHE = {}
_LAST_IN_MAPS = None


def kernel(x, W_q, scales, zeros):
    _install_axon_hooks_shim()
    from concourse.bass_utils import run_bass_kernel_spmd

    xb4, x8t, wb_full, wf_full, seed = host_prep(x, W_q, scales, zeros)

    if "nc" not in _NC_CACHE:
        _NC_CACHE["nc"] = build_bass()
    nc = _NC_CACHE["nc"]

    in_maps = []
    for c in range(N_CORES):
        lo, hi = c * N_SHARD, (c + 1) * N_SHARD
        in_maps.append(
            {
                "xb4": xb4,
                "xf4": x8t,
                "wb": np.ascontiguousarray(wb_full[:, :, lo:hi]),
                "wf": np.ascontiguousarray(wf_full[:, :, :, lo:hi]),
            }
        )

    global _LAST_IN_MAPS
    _LAST_IN_MAPS = in_maps
    res = run_bass_kernel_spmd(nc, in_maps, list(range(N_CORES)))
    psum = np.concatenate(
        [res.results[c]["out"] for c in range(N_CORES)], axis=1
    )  # [M, N] f32
    return (psum * (1.0 / SC) + seed).astype(BF16)


# revision 25
# speedup vs baseline: 1.0029x; 1.0029x over previous
"""HQQ int4 weight-only quantized linear for TRN2, 8-core tensor-parallel.

out[M, N] = x[M, K] @ dequant(W_q[N, K]).T
  dequant: w[n, k] = (q[n, k] - 8) * scales[n, k//128] + zeros[n, k//128]

Sharding: column-parallel over N (out_features) across 8 NeuronCores;
x replicated; outputs concatenated on host. No collectives.

Mixed-precision K-split (device does only matmuls; all dequant + zero-point
compensation is host-side):
  - NB=16 quant groups matmul'd in bf16: w_bf = (q-8)*s*512 (bf16), x as-is.
  - NF8=16 groups in fp8 e4m3 DoubleRow (2 groups per instruction, 2x PE
    throughput): w8 = e4m3((q-8)*s*32), x8 = e4m3(x*16) -> products carry
    the same x512 scale, so both parts accumulate in one PSUM bank.
  - PSUM evicted as raw f32; host computes out = bf16(psum/512 + seed) where
    seed = R@ (512 z).T plus group-mean quantization-error corrections
    (all exact f32 host math).
Per 128-row m-subtile: 3 n-tiles x (16 bf16 + 8 DoubleRow) = 72 matmuls
(24 "units" of 1376 rows vs 33 for all-bf16 with on-device seeds).
Startup: HAM warm-up matmuls + a DMA/k-sweep co-schedule across the two
hardware queues eliminate PE stalls (sim rel err 1.850e-2 == HW).
"""

import sys

import numpy as np
import ml_dtypes

M = 4096
K = 4096
N = 11008
GROUP = 128
N_CORES = 8
N_SHARD = N // N_CORES  # 1376
NG = K // GROUP  # 32 quant groups
NB = 16  # groups done in bf16
NF8 = NG - NB  # groups done in fp8 DoubleRow
NPAIR = NF8 // 2
M_PANEL = 256
SW = 32.0  # fp8 weight scale
SX = 16.0  # fp8 x scale
SC = SW * SX  # common PSUM scale (512)
BF16 = ml_dtypes.bfloat16
E4M3 = ml_dtypes.float8_e4m3


def _install_axon_hooks_shim():
    """antenv.axon_hooks is missing from this image; run_bass_kernel_spmd
    imports it when tracing is requested (e.g. BASS_TRACE=1). Provide the
    same ctypes-based hook trn_boot would have registered."""
    import types

    try:
        import antenv.axon_hooks  # noqa: F401

        return
    except ImportError:
        pass
    try:
        import antenv
        from trn_agent_boot.trn_boot import _ntff_profile_via_ctypes

        hook = _ntff_profile_via_ctypes("/opt/axon/libaxon_pjrt.so")
        mod = types.ModuleType("antenv.axon_hooks")
        mod._hook = hook
        mod.get_axon_ntff_profile_hook = lambda: mod._hook

        def _set(h):
            mod._hook = h

        mod.set_axon_ntff_profile_hook = _set
        sys.modules["antenv.axon_hooks"] = mod
        antenv.axon_hooks = mod
    except Exception:
        pass


def build_bass(m=M, n_shard=N_SHARD, nb=NB, npair=NPAIR, compile=True):
    import concourse.mybir as mybir
    import concourse.tile as tile
    from concourse import bacc

    P = 128
    MP = M_PANEL
    f32 = mybir.dt.float32
    bf16 = mybir.dt.bfloat16
    fp8 = mybir.dt.float8e4
    n_panels = m // MP
    nsub = MP // P  # m-subtiles per panel (2)

    nc = bacc.Bacc("TRN2", target_bir_lowering=False, debug=False)
    xb4 = nc.dram_tensor("xb4", [n_panels, P, nb, MP], bf16, kind="ExternalInput")
    xf4 = nc.dram_tensor("xf4", [n_panels, P, npair, 2, MP], fp8, kind="ExternalInput")
    wb = nc.dram_tensor("wb", [nb, P, n_shard], bf16, kind="ExternalInput")
    wf = nc.dram_tensor("wf", [npair, P, 2, n_shard], fp8, kind="ExternalInput")
    out = nc.dram_tensor("out", [m, n_shard], f32, kind="ExternalOutput")

    n_tiles = []
    st = 0
    while st < n_shard:
        nf = min(512, n_shard - st)
        n_tiles.append((st, nf))
        st += nf

    with tile.TileContext(nc) as tc:
        with (
            tc.tile_pool(name="warm", bufs=1) as warm_pool,
            tc.tile_pool(name="wbp", bufs=nb) as wb_pool,
            tc.tile_pool(name="wfp", bufs=npair) as wf_pool,
            tc.tile_pool(name="xbp", bufs=3) as xb_pool,
            tc.tile_pool(name="xfp", bufs=2) as xf_pool,
            tc.tile_pool(name="osb", bufs=2) as osb_pool,
            tc.tile_pool(name="pswm", bufs=1, space="PSUM") as pswm_pool,
            tc.tile_pool(name="psum", bufs=6, space="PSUM") as psum_pool,
        ):
            # ---- HAM warm-up: dummy matmuls with no DMA deps keep the PE
            # busy during the initial weight/x DMA wait so the clock gate is
            # at K=8/8 (2.4 GHz) when real matmuls start ----
            wrm_w = warm_pool.tile([P, P], bf16, tag="wrmw")
            wrm_x = warm_pool.tile([P, 512], bf16, tag="wrmx")
            nc.vector.memset(wrm_w[:], 0.0)
            nc.vector.memset(wrm_x[:], 0.0)
            wrm_ps = pswm_pool.tile([P, 512], f32, tag="wrmp")
            NWARM = 20
            for i in range(NWARM):
                nc.tensor.matmul(
                    wrm_ps[:], wrm_w[:], wrm_x[:],
                    start=(i == 0), stop=(i == NWARM - 1),
                )

            xb_tiles = {}
            xf_tiles = {}

            def load_panel(mp):
                xb_tiles[mp] = xb_pool.tile([P, nb, MP], bf16, tag="xb", name=f"xb{mp}")
                nc.sync.dma_start(xb_tiles[mp][:], xb4[mp])
                xf_tiles[mp] = xf_pool.tile(
                    [P, npair, 2, MP], fp8, tag="xf", name=f"xf{mp}"
                )
                nc.sync.dma_start(xf_tiles[mp][:], xf4[mp])

            # ---- DMA issue order + matching k-sweep order. Startup is
            # aggregate-HBM-bound (~10 MB of weights + panel-0 x), so the
            # sweep alternates fp8 pairs (fed by the scalar queue) with even
            # bf16 groups (fed by the sync queue) — each queue then only has
            # to sustain one 0.35 MB tile per 2.29 us — and consumes the odd
            # bf16 groups (queued behind the fp8 weights on scalar) last.
            # step schedule: two fp8 steps lead (xb0 is behind xf0 on sync),
            # then fp8/bf16-even alternate, odd bf16 groups last
            steps = [("f", 0), ("f", 1), ("b", 0)]
            for i in range(2, npair):
                steps += [("f", i), ("b", 2 * (i - 1))]
            steps += [("b", 14)]
            steps += [("b", g) for g in range(1, nb, 2)]
            assert len(steps) == nb + npair
            assert sorted(i for k, i in steps if k == "b") == list(range(nb))
            assert sorted(i for k, i in steps if k == "f") == list(range(npair))

            # Per-queue delivery schedule, derived from measured queue rates
            # (sync ~0.21 MB/us from ~8.6us, scalar ~0.24 MB/us from ~10.5us,
            # consumption one 0.35 MB tile per 1.15us from ~14.5us): scalar
            # covers steps {0,1,2} then every other step; sync carries the
            # panel-0 x tensors first, then the remaining steps' weights.
            q10_steps = {0, 1, 2} | set(range(4, len(steps), 2))

            xf_tiles[0] = xf_pool.tile([P, npair, 2, MP], fp8, tag="xf", name="xf0")
            nc.sync.dma_start(xf_tiles[0][:], xf4[0])
            xb_tiles[0] = xb_pool.tile([P, nb, MP], bf16, tag="xb", name="xb0")
            nc.sync.dma_start(xb_tiles[0][:], xb4[0])

            wb_tiles = [None] * nb
            wf_tiles = [None] * npair
            for ring_is_q10 in (True, False):
                for si, (kind, idx) in enumerate(steps):
                    if (si in q10_steps) != ring_is_q10:
                        continue
                    ring = nc.scalar if ring_is_q10 else nc.sync
                    if kind == "f":
                        wt = wf_pool.tile(
                            [P, 2, n_shard], fp8, tag="wf", name=f"wf{idx}"
                        )
                        ring.dma_start(wt[:], wf[idx])
                        wf_tiles[idx] = wt
                    else:
                        wt = wb_pool.tile(
                            [P, n_shard], bf16, tag="wb", name=f"wb{idx}"
                        )
                        ring.dma_start(wt[:], wb[idx])
                        wb_tiles[idx] = wt

            def mm_sweep_one(pss, mp, ms, si):
                """One k-step (steps[si]) across the 3 n-tiles."""
                kind, idx = steps[si]
                first, last = si == 0, si == len(steps) - 1
                if kind == "f":
                    lhsT = xf_tiles[mp][:, idx, :, ms * P : (ms + 1) * P]
                    for j, (st, nf) in enumerate(n_tiles):
                        nc.tensor.matmul(
                            pss[j],
                            lhsT,
                            wf_tiles[idx][:, :, st : st + nf],
                            start=first,
                            stop=last,
                            perf_mode=mybir.MatmulPerfMode.DoubleRow,
                        )
                else:
                    lhsT = xb_tiles[mp][:, idx, ms * P : (ms + 1) * P]
                    for j, (st, nf) in enumerate(n_tiles):
                        nc.tensor.matmul(
                            pss[j],
                            lhsT,
                            wb_tiles[idx][:, st : st + nf],
                            start=first,
                            stop=last,
                        )

            def evict(psums, ms_abs):
                osb = osb_pool.tile([P, n_shard], f32, tag="osb")
                for j, (st, nf) in enumerate(n_tiles):
                    nc.any.tensor_copy(osb[:, st : st + nf], psums[j])
                m0 = ms_abs * P
                nc.scalar.dma_start(out[m0 : m0 + P, :], osb[:])

            nsteps = nb + npair

            def emit_panel_k_outer(mp):
                # both m-subtiles' k-sweeps interleaved: 6 open psum banks;
                # matmuls chase the weight DMAs on the first panels.
                pss = []
                for ms in range(nsub):
                    row = []
                    for j, (st, nf) in enumerate(n_tiles):
                        ps = psum_pool.tile([P, 512], f32, tag="ps", name="psA")[:, :nf]
                        row.append(ps)
                    pss.append(row)
                for g in range(nsteps):
                    for ms in range(nsub):
                        mm_sweep_one(pss[ms], mp, ms, g)
                for ms in range(nsub):
                    evict(pss[ms], mp * nsub + ms)

            def emit_panel_ms_inner(mp):
                for ms in range(nsub):
                    psums = []
                    for j, (st, nf) in enumerate(n_tiles):
                        ps = psum_pool.tile([P, 512], f32, tag="ps", name="psB")[:, :nf]
                        psums.append(ps)
                    for g in range(nsteps):
                        mm_sweep_one(psums, mp, ms, g)
                    evict(psums, mp * nsub + ms)

            def emit_last_ms_j_outer(mp, ms):
                # j-outer so each n-tile's k-sweep finishes early and its
                # evict + out-DMA overlap the remaining n-tiles' matmuls —
                # shortens the end-of-kernel tail.
                xb_t, xf_t = xb_tiles[mp], xf_tiles[mp]
                osb = osb_pool.tile([P, n_shard], f32, tag="osb", name="osbL")
                m0 = (mp * nsub + ms) * P
                for j, (st, nf) in enumerate(n_tiles):
                    ps = psum_pool.tile([P, 512], f32, tag="ps", name="psC")[:, :nf]
                    for g in range(nb):
                        nc.tensor.matmul(
                            ps,
                            xb_t[:, g, ms * P : (ms + 1) * P],
                            wb_tiles[g][:, st : st + nf],
                            start=(g == 0),
                            stop=False,
                        )
                    for p in range(npair):
                        nc.tensor.matmul(
                            ps,
                            xf_t[:, p, :, ms * P : (ms + 1) * P],
                            wf_tiles[p][:, :, st : st + nf],
                            start=False,
                            stop=(p == npair - 1),
                            perf_mode=mybir.MatmulPerfMode.DoubleRow,
                        )
                    nc.any.tensor_copy(osb[:, st : st + nf], ps)
                    nc.scalar.dma_start(
                        out[m0 : m0 + P, st : st + nf], osb[:, st : st + nf]
                    )

            for mp in range(n_panels):
                if mp > 0:
                    load_panel(mp)
                if mp < 2:
                    emit_panel_k_outer(mp)
                elif mp < n_panels - 1:
                    emit_panel_ms_inner(mp)
                else:
                    # last panel: normal first subtile, j-outer last subtile
                    psums = []
                    for j, (st, nf) in enumerate(n_tiles):
                        ps = psum_pool.tile([P, 512], f32, tag="ps", name="psB")[:, :nf]
                        psums.append(ps)
                    for g in range(nsteps):
                        mm_sweep_one(psums, mp, 0, g)
                    evict(psums, mp * nsub)
                    emit_last_ms_j_outer(mp, 1)

    if compile:
        nc.compile()
    return nc


def host_prep(x, W_q, scales, zeros):
    """Host-side layout/quantization prep. Returns per-core input maps'
    building blocks plus the f32 seed (zero-point + quant-error-mean
    compensation) to add to the device output."""
    n_panels = M // M_PANEL
    x = np.asarray(x)
    xf32 = x.astype(np.float32)
    sf = np.asarray(scales).astype(np.float32)
    zf = np.asarray(zeros).astype(np.float32)

    # dequantized (scaled) weight without zeros, transposed to [K, N]
    q = np.asarray(W_q).astype(np.float32) - 8.0  # [N, K]
    w_sT = (q.reshape(N, NG, GROUP) * sf[:, :, None]).reshape(N, K).T  # [K, N]
    w_sT = np.ascontiguousarray(w_sT)

    kb = NB * GROUP  # bf16 k-columns: [0, kb); fp8: [kb, K)
    w_bfT = (w_sT[:kb] * SC).astype(BF16)  # [kb, N]
    w_f8T = (w_sT[kb:] * SW).astype(E4M3)  # [K-kb, N]

    # x tiles: [panel, p, group, m_in_panel]
    xt = np.ascontiguousarray(
        x.reshape(n_panels, M_PANEL, NG, GROUP).transpose(0, 3, 2, 1)
    )  # [np, 128, ng, MP] bf16
    xb4 = np.ascontiguousarray(xt[:, :, :NB, :])
    x8 = (xf32 * SX).astype(E4M3)  # [M, K]
    x8t = np.ascontiguousarray(
        x8.reshape(n_panels, M_PANEL, NG, GROUP)
        .transpose(0, 3, 2, 1)[:, :, NB:, :]
        .reshape(n_panels, GROUP, NPAIR, 2, M_PANEL)
    )

    # weight tensors in device layout
    wb_full = np.ascontiguousarray(w_bfT.reshape(NB, GROUP, N))  # [nb, 128, N]
    wf_full = np.ascontiguousarray(
        w_f8T.reshape(NPAIR, 2, GROUP, N).transpose(0, 2, 1, 3)
    )  # [npair, 128, 2, N]

    # ---- host seed: zero-point compensation + quant-error group means ----
    R = xf32.reshape(M, NG, GROUP).sum(-1)  # [M, ng] exact group sums
    e_x = xf32 - x8.astype(np.float32) / SX  # x quant residual
    Ex = e_x.reshape(M, NG, GROUP).sum(-1)[:, NB:]  # [M, nf8]

    w_q_errT = w_sT[kb:] - w_f8T.astype(np.float32) / SW  # [kf, N]
    ebar8 = w_q_errT.reshape(NF8, GROUP, N).mean(1).T  # [N, nf8]
    w_bf_errT = w_sT[:kb] - w_bfT.astype(np.float32) / SC  # [kb, N]
    ebarb = w_bf_errT.reshape(NB, GROUP, N).mean(1).T  # [N, nb]
    wbar8 = w_sT[kb:].reshape(NF8, GROUP, N).mean(1).T  # [N, nf8]

    zc = zf.copy()  # [N, ng]
    zc[:, :NB] += ebarb
    zc[:, NB:] += ebar8
    seed = R @ zc.T + Ex @ (wbar8 - ebar8).T  # [M, N] f32
    return xb4, x8t, wb_full, wf_full, seed


_NC_CACHE = {}
_LAST_IN_MAPS = None


def kernel(x, W_q, scales, zeros):
    _install_axon_hooks_shim()
    from concourse.bass_utils import run_bass_kernel_spmd

    xb4, x8t, wb_full, wf_full, seed = host_prep(x, W_q, scales, zeros)

    if "nc" not in _NC_CACHE:
        _NC_CACHE["nc"] = build_bass()
    nc = _NC_CACHE["nc"]

    in_maps = []
    for c in range(N_CORES):
        lo, hi = c * N_SHARD, (c + 1) * N_SHARD
        in_maps.append(
            {
                "xb4": xb4,
                "xf4": x8t,
                "wb": np.ascontiguousarray(wb_full[:, :, lo:hi]),
                "wf": np.ascontiguousarray(wf_full[:, :, :, lo:hi]),
            }
        )

    global _LAST_IN_MAPS
    _LAST_IN_MAPS = in_maps
    res = run_bass_kernel_spmd(nc, in_maps, list(range(N_CORES)))
    psum = np.concatenate(
        [res.results[c]["out"] for c in range(N_CORES)], axis=1
    )  # [M, N] f32
    return (psum * (1.0 / SC) + seed).astype(BF16)
